# revision 30
# baseline (speedup 1.0000x reference)
"""Fused multi-head attention (B=2, N=2048, C=1024, H=16) on 8 TRN2 NeuronCores.

Sharding: core = (b, g) with b = batch (2) and g = head-group of 4 heads (4).
Each core computes, for its batch and 4 heads:
    qkv slice -> per-head softmax attention -> out-proj partial (row-parallel).
Host sums the 4 per-head-group proj partials per batch and adds b_proj.

Device algorithm (per core), matmuls in bf16:
  qkT = (x @ Wqk)^T   [q/k feats on partitions, 2048 tokens]
  v   = x @ Wv        [2048 tokens, 4*64] (+ ones column per head)
  per (head pair hp, 512-row chunk rc), 8 kc2 steps over 16 key chunks:
    S^T tiles = matmul(lhsT=kTp_h, rhs=q-chunk)  [128 keys, 512 rows]
      kTp is K=128 zero-padded per head (even head rows 0:64, odd 64:128)
    expST = exp(S^T/8)  (ScalarE, PSUM->SBUF, [128,1024] per head per step)
    outT[65, rows] += [v_h|1]^T-matmul expST  (row 64 = denominator)
    outT[0:64] *= 1/denominator  (DVE approx-recip, GpSimd bcast, DVE mult)
  partial = out^T-matmul Wp -> DMA out

Scheduling (the perf-critical part; the kernel is PE-streaming-bound at
~164us of matmul cycles, so everything aims at keeping the PE dense):
  - All PE work is emitted as one stream of 64 attention steps with
    qk/v/proj "fill" units spread between them by a static EDF scheduler,
    so the PE never idles (idle >3.4us triggers HAM re-throttle to half
    clock) and never blocks at head-of-line on ScalarE.
  - PV consumes the PREVIOUS step's exp output (1-step software pipeline
    skew) so the PE does not wait for the exp it just issued. Exception:
    the last step of each block emits its PV in-step (after extra fills)
    so the block's psum accumulators free at the block boundary.
  - Fills are interleaved into 4 gaps between ST/PV half-groups: with
    fillps bufs=1, back-to-back fills stall on the previous fill's
    PSUM->SBUF copy; >=2 attention half-groups between fills hide it.
  - DMA is issued in large rearranged pieces ordered so the first ST can
    start a few us in (k01 weight cols + xT token-chunk 0 first).
  - PSUM budget (8 banks): stps 2x[128,1024] (4) + pvps 3x[128,512] (3)
    + fillps 1x[128,512] (1).
Measured on TRN2: ~211us (from a 258us tuned baseline; engine-active:
PE ~181us = bottleneck, ACT ~137us, DVE ~79us).
Notes from failed experiments (do not retry):
  - fp8 anywhere in the data path fails the 2e-2 rel-err budget: softmax
    weight noise transfers ~1:1 to the output (random v does not average
    it down); q/k fp8 -> 3.4e-2, v fp8 -> 1.9e-2, ex fp8 -> 2e-2.
  - fp8 DoubleRow issues at the same ns/output-col as bf16 (2 fp8/cycle
    bus) so hi/lo-split DR (3 terms) is 1.5x SLOWER than bf16.
  - LDWEIGHTS are fully shadow-loaded behind the previous matmul (issue
    rate = streaming rate, even at 109ns); reducing matmul count or
    weight reloads buys nothing.
  - PE warm-up dummy matmuls before the DMAs land just idle-reset the
    p-state ramp and delay the prefix.
"""

import os
from collections import defaultdict

import numpy as np

import concourse.bass as bass
import concourse.mybir as mybir
import concourse.tile as tile
from concourse import bacc
from concourse.bass_utils import run_bass_kernel_spmd

B, N, C = 2, 2048, 1024
HC = 4  # heads per core
D = 64
NCORES = 8
KC = C // 128  # 8 contraction chunks for qkv matmuls
SCALE = D**-0.5  # 0.125

MM_DT = os.environ.get("ATTN_MM_DT", "bf16")
# per-step PE target (ns) the fill scheduler aims for
TARGET_NS = int(os.environ.get("ATTN_TARGET_NS", "2650"))
RECIP = os.environ.get("ATTN_RECIP", "approx")  # approx | exact

BLOCKS = [(0, 0), (0, 1), (1, 0), (0, 2), (1, 1), (0, 3), (1, 2), (1, 3)]


def _np_in_dtype():
    if MM_DT == "bf16":
        import ml_dtypes

        return np.dtype(ml_dtypes.bfloat16)
    return np.dtype(np.float32)


def _prep(a):
    a = np.ascontiguousarray(a)
    return a.astype(_np_in_dtype())


def build_nc():
    f32 = mybir.dt.float32
    in_dt = {"bf16": mybir.dt.bfloat16, "f32": mybir.dt.float32}[MM_DT]

    nc = bacc.Bacc("TRN2", target_bir_lowering=False, debug=False, num_devices=NCORES)
    xT_d = nc.dram_tensor("xT", [C, N], in_dt, kind="ExternalInput").ap()
    wqk_d = nc.dram_tensor("wqk", [C, 2 * HC * D], in_dt, kind="ExternalInput").ap()
    wv_d = nc.dram_tensor("wv", [C, HC * D], in_dt, kind="ExternalInput").ap()
    wp_d = nc.dram_tensor("wp", [HC * D, C], in_dt, kind="ExternalInput").ap()
    out_d = nc.dram_tensor("out", [N, C], f32, kind="ExternalOutput").ap()

    with tile.TileContext(nc) as tc:
        with (
            tc.tile_pool(name="const", bufs=1) as const,
            tc.tile_pool(name="ex", bufs=8) as expool,
            tc.tile_pool(name="den", bufs=8) as dpool,
            tc.tile_pool(name="stage", bufs=4) as stage,
            tc.tile_pool(name="stps", bufs=2, space="PSUM") as stps,
            tc.tile_pool(name="pvps", bufs=3, space="PSUM") as pvps,
            tc.tile_pool(name="fillps", bufs=1, space="PSUM") as fillps,
        ):
            # persistent tiles
            qkT_sb = const.tile([128, 2, N], in_dt, tag="qkT")
            kTp_sb = const.tile([128, HC, N], in_dt, tag="kTp")
            v_sb = const.tile([128, 16, HC, D + 1], in_dt, tag="v")
            wp_sb = const.tile([128, 2, C], in_dt, tag="wp")
            outT_sb = const.tile([128, 2, N], in_dt, tag="outT")
            xT_sb = const.tile([128, KC, N], in_dt, tag="xT")
            wqk_sb = const.tile([128, KC, 2 * HC * D], in_dt, tag="wqk")
            wv_sb = const.tile([128, KC, HC * D], in_dt, tag="wv")

            # ---- one-time fills first (no deps; engines warm up) ----
            # kTp zero halves: head even -> rows 64:128 zero, odd -> 0:64
            for h in range(HC):
                zb = 64 if h % 2 == 0 else 0
                nc.vector.memset(kTp_sb[zb : zb + 64, h, :], 0.0)
            ones_f32 = const.tile([128, 16, HC, 1], f32, tag="ones")
            nc.vector.memset(ones_f32[:], 1.0)
            nc.vector.tensor_copy(v_sb[:, :, :, D : D + 1], ones_f32[:])

            # ---- DMAs: 10 large pieces, ordered for earliest first ST ----
            # wqk col chunks: 0=q01, 1=q23, 2=k01, 3=k23
            def dma_wqk(mf, k0=0, k1=KC):
                nc.sync.dma_start(
                    wqk_sb[:, k0:k1, mf * 128 : (mf + 1) * 128],
                    wqk_d[k0 * 128 : k1 * 128, mf * 128 : (mf + 1) * 128].rearrange(
                        "(kc p) c -> p kc c", p=128
                    ),
                )

            def dma_xT(nt, k0=0, k1=KC):
                nts = slice(nt * 512, (nt + 1) * 512)
                nc.sync.dma_start(
                    xT_sb[:, k0:k1, nts],
                    xT_d[k0 * 128 : k1 * 128, nts].rearrange(
                        "(kc p) c -> p kc c", p=128
                    ),
                )

            # first pieces sliced fine: the first qk matmuls (subtile deps)
            # start as soon as the first ~0.4MB lands
            dma_wqk(2, 0, 4)
            dma_xT(0, 0, 2)
            dma_xT(0, 2, 4)
            dma_wqk(2, 4, 8)
            dma_xT(0, 4, 6)
            dma_xT(0, 6, 8)
            dma_wqk(0)
            nc.sync.dma_start(
                wv_sb[:], wv_d.rearrange("(kc p) c -> p kc c", p=128)
            )
            dma_xT(1)
            dma_xT(2)
            dma_xT(3)
            dma_wqk(1)
            dma_wqk(3)
            for c2 in range(2):
                nc.sync.dma_start(wp_sb[:, c2, :], wp_d[c2 * 128 : (c2 + 1) * 128, :])

            # ---- emission helpers ----
            def qk_chunk(mf, nt, ps_pool=None):
                """(x @ Wqk)^T feat chunk mf, token chunk nt. 8 matmuls."""
                if ps_pool is None:
                    ps = fillps.tile([128, 512], f32, tag="fp", name="fp")
                else:
                    ps = ps_pool.tile([128, 512], f32, tag="st", name="st")
                for kc in range(KC):
                    nc.tensor.matmul(
                        ps,
                        wqk_sb[:, kc, mf * 128 : (mf + 1) * 128],
                        xT_sb[:, kc, nt * 512 : (nt + 1) * 512],
                        start=(kc == 0),
                        stop=(kc == KC - 1),
                    )
                nts = slice(nt * 512, (nt + 1) * 512)
                if mf < 2:
                    nc.vector.tensor_copy(qkT_sb[:, mf, nts], ps)
                else:
                    h0, h1 = 2 * (mf - 2), 2 * (mf - 2) + 1
                    nc.vector.tensor_copy(kTp_sb[0:64, h0, nts], ps[0:64, :])
                    nc.vector.tensor_copy(kTp_sb[64:128, h1, nts], ps[64:128, :])

            def v_chunk(t):
                """v = x @ Wv for token(=key) chunk t, all heads. 8 matmuls."""
                ps = fillps.tile([128, 512], f32, tag="fp", name="fp")[:, : HC * D]
                for kc in range(KC):
                    nc.tensor.matmul(
                        ps,
                        xT_sb[:, kc, t * 128 : (t + 1) * 128],
                        wv_sb[:, kc, :],
                        start=(kc == 0),
                        stop=(kc == KC - 1),
                    )
                nc.vector.tensor_copy(
                    v_sb[:, t, :, 0:D], ps.rearrange("p (h d) -> p h d", h=HC)
                )

            def proj_chunk(t, nf, ps_pool=None):
                """partial[t*128:(t+1)*128, nf*512:(nf+1)*512] = out @ Wp."""
                if ps_pool is None:
                    ps = fillps.tile([128, 512], f32, tag="fp", name="fp")
                else:
                    ps = ps_pool.tile([128, 512], f32, tag="pv", name="pv")
                for c2 in range(2):
                    nc.tensor.matmul(
                        ps,
                        outT_sb[:, c2, t * 128 : (t + 1) * 128],
                        wp_sb[:, c2, nf * 512 : (nf + 1) * 512],
                        start=(c2 == 0),
                        stop=(c2 == 1),
                    )
                sg = stage.tile([128, 512], f32, tag="sg", name="sg")
                nc.vector.tensor_copy(sg, ps)
                nc.sync.dma_start(
                    out_d[t * 128 : (t + 1) * 128, nf * 512 : (nf + 1) * 512], sg
                )

            # ---- static fill schedule (EDF with per-step budget) ----
            # unit: (cost_ns, release_step, deadline_step, emit_fn)
            # deadlines are STRICT: the unit's consumer is at step dl+1 (or
            # later), so placing at step <= dl can never put a fill after
            # the PE instruction that waits on it (emission-order deadlock).
            units = []
            qk_dl = {
                (2, 1): 1, (2, 2): 3, (2, 3): 5,    # k01 feeds ST(2i) in (0,0)
                (0, 1): 7, (0, 2): 23, (0, 3): 39,  # q01 feeds (0,rc) first ST
                (3, 0): 15, (3, 1): 17, (3, 2): 19, (3, 3): 21,  # k23
                (1, 0): 15, (1, 1): 31, (1, 2): 47, (1, 3): 55,  # q23
            }
            # stagger far-deadline releases so the EDF doesn't drain the fill
            # supply early and starve mid-kernel steps
            qk_rel = {
                (2, 1): 0, (2, 2): 0, (2, 3): 0,
                (0, 1): 0, (0, 2): 12, (0, 3): 33,
                (3, 0): 5, (3, 1): 5, (3, 2): 6, (3, 3): 6,
                (1, 0): 5, (1, 1): 20, (1, 2): 40, (1, 3): 49,
            }
            for (mf, nt), dl in qk_dl.items():
                units.append(
                    (1706, qk_rel[(mf, nt)], dl,
                     lambda mf=mf, nt=nt: qk_chunk(mf, nt))
                )
            for t in range(16):
                dl = min(t // 2, 6)  # PV(kc=t) at step t//2+1 (in-step at 7)
                units.append((853, 0, dl, lambda t=t: v_chunk(t)))
            # proj: releases staggered across the supply window; rc2 keeps 4
            # units reserved for the tail (rel 64) so the PE has warm work
            # during the last block's rescale chain
            for rc in range(4):
                for i, (t, nf) in enumerate(
                    (t, nf) for t in range(4 * rc, 4 * rc + 4) for nf in range(2)
                ):
                    rel, dl = {0: 25 + i, 1: 41 + i, 2: 57 + i, 3: 66}[rc], 99
                    if rc == 2 and i >= 4:
                        # 2 units pinned to step 63 (cover the last exp wait),
                        # 2 go to the tail to overlap the final rescale
                        rel, dl = (63, 63) if i < 6 else (64, 99)
                    units.append(
                        (427, rel, dl,
                         lambda t=t, nf=nf, **kw: proj_chunk(t, nf, **kw))
                    )

            # attention PE cost per step (ns): first step of a block has no
            # PV (skew), last step has ST + 2 PVs
            def attn_cost(s):
                k = s % 8
                return 853 if k == 0 else (2559 if k == 7 else 1706)

            slots = defaultdict(list)  # step -> [unit...]
            pending = sorted(units, key=lambda u: (u[2], u[1]))
            placed_cost = defaultdict(int)
            unplaced = list(pending)
            for s in range(64):
                ready = [u for u in unplaced if u[1] <= s]
                ready.sort(key=lambda u: u[2])
                # forced: deadline at or before this step
                while ready and ready[0][2] <= s:
                    u = ready.pop(0)
                    slots[s].append(u)
                    placed_cost[s] += u[0]
                    unplaced.remove(u)
                while ready and placed_cost[s] + attn_cost(s) < TARGET_NS:
                    u = ready.pop(0)
                    slots[s].append(u)
                    placed_cost[s] += u[0]
                    unplaced.remove(u)
            tail_units = unplaced

            # ---- attention blocks ----
            pv_tiles = {}
            ex_pend = []  # (heads, kcs, pv, ex{h}) awaiting PV emission

            def emit_st_head(hp, rc, kc2, h, stp):
                """Both j-halves of one head's S^T step, then its exp."""
                for j in range(2):
                    kc = 2 * kc2 + j
                    nc.tensor.matmul(
                        stp[h][:, j * 512 : (j + 1) * 512],
                        kTp_sb[:, h, kc * 128 : (kc + 1) * 128],
                        qkT_sb[:, hp, rc * 512 : (rc + 1) * 512],
                        start=True,
                        stop=True,
                    )
                ex = expool.tile([128, 1024], in_dt, tag="ex", name="ex")
                nc.scalar.activation(
                    ex, stp[h], mybir.ActivationFunctionType.Exp, scale=SCALE
                )
                return ex

            def emit_pv_head(pv, h, kc2, ex):
                for j in range(2):
                    kc = 2 * kc2 + j
                    nc.tensor.matmul(
                        pv[h][: D + 1, :],
                        v_sb[:, kc, h, :],
                        ex[:, j * 512 : (j + 1) * 512],
                        start=(kc == 0),
                        stop=(kc == 15),
                    )

            def emit_rescale(hp, rc, heads, pv):
                recs, rbcs, dens = {}, {}, {}
                for h in heads:
                    if RECIP == "approx":
                        # recip_approx_fast breaks on partition-offset input;
                        # stage the denominator row at partition 0 first.
                        dens[h] = dpool.tile([1, 512], f32, tag="dcp", name="dcp")
                        nc.vector.tensor_copy(dens[h], pv[h][D : D + 1, :])
                for h in heads:
                    recs[h] = dpool.tile([1, 512], f32, tag="den", name="den")
                    if RECIP == "approx":
                        nc.vector.reciprocal_approx_fast(recs[h], dens[h])
                    else:
                        nc.vector.reciprocal(recs[h], pv[h][D : D + 1, :])
                for h in heads:
                    rbcs[h] = dpool.tile([64, 512], f32, tag="rbc", name="rbc")
                    nc.gpsimd.partition_broadcast(rbcs[h], recs[h])
                for h in heads:
                    hb = (h % 2) * 64
                    nc.vector.tensor_tensor(
                        out=outT_sb[hb : hb + 64, hp, rc * 512 : (rc + 1) * 512],
                        in0=pv[h][0:D, :],
                        in1=rbcs[h][:],
                        op=mybir.AluOpType.mult,
                    )

            # prefix: minimum to start attention (second chunk on the idle
            # stps pool so it doesn't serialize on fillps behind the first)
            qk_chunk(2, 0)               # k01 keys 0:512
            qk_chunk(0, 0, ps_pool=stps)  # q01 rows 0:512

            def pop_pv(pv):
                h, k2, ex = ex_pend.pop(0)
                emit_pv_head(pv, h, k2, ex)

            rescale_after = None  # (hp, rc, heads, pv) from previous block
            for s in range(64):
                bi, kc2 = s // 8, s % 8
                hp, rc = BLOCKS[bi]
                heads = (2 * hp, 2 * hp + 1)
                if kc2 == 0:
                    pv_tiles[(hp, rc)] = {
                        h: pvps.tile([128, 512], f32, tag="pv", name="pv")
                        for h in heads
                    }
                pv = pv_tiles[(hp, rc)]
                # round-robin fills into 4 gaps between ST/PV half-groups so
                # no two fills are adjacent (fillps bufs=1: back-to-back
                # fills stall on the PSUM->SBUF copy of the previous one)
                fp = [[], [], [], []]
                if kc2 == 0:
                    # block boundary: ST waits on the previous step's exp via
                    # stp recycle — put all fills before the STs
                    fills = slots.get(s, [])
                    fp[0], fp[1] = fills[0::2], fills[1::2]
                else:
                    for i, u in enumerate(slots.get(s, [])):
                        fp[i % 4].append(u)
                stp = {
                    h: stps.tile([128, 1024], f32, tag="st", name="st") for h in heads
                }
                for u in fp[0]:
                    u[3]()
                ex_pend.append(
                    (heads[0], kc2, emit_st_head(hp, rc, kc2, heads[0], stp))
                )
                for u in fp[1]:
                    u[3]()
                ex_pend.append(
                    (heads[1], kc2, emit_st_head(hp, rc, kc2, heads[1], stp))
                )
                for u in fp[2]:
                    u[3]()
                if kc2 == 0:
                    # no PVs this step (block boundary); finish prev block
                    if rescale_after is not None:
                        emit_rescale(*rescale_after)
                        rescale_after = None
                    for u in fp[3]:
                        u[3]()
                else:
                    pop_pv(pv)
                    for u in fp[3]:
                        u[3]()
                    pop_pv(pv)
                    if kc2 == 7:
                        pop_pv(pv)  # PV(kc2=7) in-step
                        pop_pv(pv)
                        rescale_after = (hp, rc, heads, pv)
            # tail: last block rescale, then remaining fills (proj rc=3).
            # The attention psum pools are free now — rotate proj psums
            # across pvps+fillps so the units pipeline instead of each
            # matmul waiting on the previous unit's PSUM->SBUF copy.
            # two reserved proj2 units first (independent of the last
            # rescale) keep the PE warm while the rescale chain runs
            for u in tail_units[:1]:
                u[3]()  # fillps
            for u in tail_units[1:2]:
                u[3](ps_pool=pvps)
            emit_rescale(*rescale_after)
            for u in tail_units[2:]:
                u[3](ps_pool=pvps)
    nc.compile()
    return nc


def make_in_maps(x, w_qkv, w_proj):
    in_maps = []
    for core in range(NCORES):
        b, g = core // 4, core % 4
        qs = slice(g * 256, (g + 1) * 256)
        in_maps.append(
            {
                "xT": _prep(x[b].T),
                "wqk": _prep(
                    np.concatenate(
                        [w_qkv[:, qs], w_qkv[:, C + g * 256 : C + (g + 1) * 256]],
                        axis=1,
                    )
                ),
                "wv": _prep(w_qkv[:, 2 * C + g * 256 : 2 * C + (g + 1) * 256]),
                "wp": _prep(w_proj[qs, :]),
            }
        )
    return in_maps


def run_hw(x, w_qkv, w_proj, b_proj, trace=False):
    """Returns (full output [2, 2048, 1024] f32, exec_time_ns or None)."""
    in_maps = make_in_maps(x, w_qkv, w_proj)
    nc = build_nc()
    r = run_bass_kernel_spmd(nc, in_maps, core_ids=list(range(NCORES)), trace=trace)
    full = np.zeros((B, N, C), np.float32)
    for core in range(NCORES):
        full[core // 4] += r.results[core]["out"]
    full += np.asarray(b_proj, np.float32)[None, None, :]
    return full, r.exec_time_ns


def kernel(**inputs):
    x = np.asarray(inputs["x"], np.float32)
    w_qkv = np.asarray(inputs["w_qkv"], np.float32)
    w_proj = np.asarray(inputs["w_proj"], np.float32)
    b_proj = np.asarray(inputs["b_proj"], np.float32)
    out, _ = run_hw(x, w_qkv, w_proj, b_proj, trace=False)
    return out


# revision 33
# speedup vs baseline: 1.0040x; 1.0040x over previous
"""Fused multi-head attention (B=2, N=2048, C=1024, H=16) on 8 TRN2 NeuronCores.

Sharding: core = (b, g) with b = batch (2) and g = head-group of 4 heads (4).
Each core computes, for its batch and 4 heads:
    qkv slice -> per-head softmax attention -> out-proj partial (row-parallel).
Host sums the 4 per-head-group proj partials per batch and adds b_proj.

Device algorithm (per core), matmuls in bf16:
  qkT = (x @ Wqk)^T   [q/k feats on partitions, 2048 tokens]
  v   = x @ Wv        [2048 tokens, 4*64] (+ ones column per head)
  per (head pair hp, 512-row chunk rc), 8 kc2 steps over 16 key chunks:
    S^T tiles = matmul(lhsT=kTp_h, rhs=q-chunk)  [128 keys, 512 rows]
      kTp is K=128 zero-padded per head (even head rows 0:64, odd 64:128)
    expST = exp(S^T/8)  (ScalarE, PSUM->SBUF, [128,1024] per head per step)
    outT[65, rows] += [v_h|1]^T-matmul expST  (row 64 = denominator)
    outT[0:64] *= 1/denominator  (DVE approx-recip, GpSimd bcast, DVE mult)
  partial = out^T-matmul Wp -> DMA out

Scheduling (the perf-critical part; the kernel is PE-streaming-bound at
~164us of matmul cycles, so everything aims at keeping the PE dense):
  - All PE work is emitted as one stream of 64 attention steps with
    qk/v/proj "fill" units spread between them by a static EDF scheduler,
    so the PE never idles (idle >3.4us triggers HAM re-throttle to half
    clock) and never blocks at head-of-line on ScalarE.
  - PV consumes the PREVIOUS step's exp output (1-step software pipeline
    skew) so the PE does not wait for the exp it just issued. Exception:
    the last step of each block emits its PV in-step (after extra fills)
    so the block's psum accumulators free at the block boundary.
  - Fills are interleaved into 4 gaps between ST/PV half-groups: with
    fillps bufs=1, back-to-back fills stall on the previous fill's
    PSUM->SBUF copy; >=2 attention half-groups between fills hide it.
  - DMA is issued in large rearranged pieces ordered so the first ST can
    start a few us in (k01 weight cols + xT token-chunk 0 first).
  - PSUM budget (8 banks): stps 2x[128,1024] (4) + pvps 3x[128,512] (3)
    + fillps 1x[128,512] (1).
Measured on TRN2: ~211us (from a 258us tuned baseline; engine-active:
PE ~181us = bottleneck, ACT ~137us, DVE ~79us).
Notes from failed experiments (do not retry):
  - fp8 anywhere in the data path fails the 2e-2 rel-err budget: softmax
    weight noise transfers ~1:1 to the output (random v does not average
    it down); q/k fp8 -> 3.4e-2, v fp8 -> 1.9e-2, ex fp8 -> 2e-2.
  - fp8 DoubleRow issues at the same ns/output-col as bf16 (2 fp8/cycle
    bus) so hi/lo-split DR (3 terms) is 1.5x SLOWER than bf16.
  - LDWEIGHTS are fully shadow-loaded behind the previous matmul (issue
    rate = streaming rate, even at 109ns); reducing matmul count or
    weight reloads buys nothing.
  - PE warm-up dummy matmuls before the DMAs land just idle-reset the
    p-state ramp and delay the prefix.
"""

import os
from collections import defaultdict

import numpy as np

import concourse.bass as bass
import concourse.mybir as mybir
import concourse.tile as tile
from concourse import bacc
from concourse.bass_utils import run_bass_kernel_spmd

B, N, C = 2, 2048, 1024
HC = 4  # heads per core
D = 64
NCORES = 8
KC = C // 128  # 8 contraction chunks for qkv matmuls
SCALE = D**-0.5  # 0.125

MM_DT = os.environ.get("ATTN_MM_DT", "bf16")
# per-step PE target (ns) the fill scheduler aims for
TARGET_NS = int(os.environ.get("ATTN_TARGET_NS", "2650"))
RECIP = os.environ.get("ATTN_RECIP", "approx")  # approx | exact

BLOCKS = [(0, 0), (0, 1), (1, 0), (0, 2), (1, 1), (0, 3), (1, 2), (1, 3)]


def _np_in_dtype():
    if MM_DT == "bf16":
        import ml_dtypes

        return np.dtype(ml_dtypes.bfloat16)
    return np.dtype(np.float32)


def _prep(a):
    a = np.ascontiguousarray(a)
    return a.astype(_np_in_dtype())


def build_nc():
    f32 = mybir.dt.float32
    in_dt = {"bf16": mybir.dt.bfloat16, "f32": mybir.dt.float32}[MM_DT]

    nc = bacc.Bacc("TRN2", target_bir_lowering=False, debug=False, num_devices=NCORES)
    xT_d = nc.dram_tensor("xT", [C, N], in_dt, kind="ExternalInput").ap()
    wqk_d = nc.dram_tensor("wqk", [C, 2 * HC * D], in_dt, kind="ExternalInput").ap()
    wv_d = nc.dram_tensor("wv", [C, HC * D], in_dt, kind="ExternalInput").ap()
    wp_d = nc.dram_tensor("wp", [HC * D, C], in_dt, kind="ExternalInput").ap()
    out_d = nc.dram_tensor("out", [N, C], f32, kind="ExternalOutput").ap()

    with tile.TileContext(nc) as tc:
        with (
            tc.tile_pool(name="const", bufs=1) as const,
            tc.tile_pool(name="ex", bufs=8) as expool,
            tc.tile_pool(name="den", bufs=8) as dpool,
            tc.tile_pool(name="stage", bufs=4) as stage,
            tc.tile_pool(name="stps", bufs=2, space="PSUM") as stps,
            tc.tile_pool(name="pvps", bufs=3, space="PSUM") as pvps,
            tc.tile_pool(name="fillps", bufs=1, space="PSUM") as fillps,
        ):
            # persistent tiles
            qkT_sb = const.tile([128, 2, N], in_dt, tag="qkT")
            kTp_sb = const.tile([128, HC, N], in_dt, tag="kTp")
            v_sb = const.tile([128, 16, HC, D + 1], in_dt, tag="v")
            wp_sb = const.tile([128, 2, C], in_dt, tag="wp")
            outT_sb = const.tile([128, 2, N], in_dt, tag="outT")
            xT_sb = const.tile([128, KC, N], in_dt, tag="xT")
            wqk_sb = const.tile([128, KC, 2 * HC * D], in_dt, tag="wqk")
            wv_sb = const.tile([128, KC, HC * D], in_dt, tag="wv")

            # ---- one-time fills first (no deps; engines warm up) ----
            # kTp zero halves: head even -> rows 64:128 zero, odd -> 0:64
            for h in range(HC):
                zb = 64 if h % 2 == 0 else 0
                nc.vector.memset(kTp_sb[zb : zb + 64, h, :], 0.0)
            ones_f32 = const.tile([128, 16, HC, 1], f32, tag="ones")
            nc.vector.memset(ones_f32[:], 1.0)
            nc.vector.tensor_copy(v_sb[:, :, :, D : D + 1], ones_f32[:])

            # ---- DMAs: 10 large pieces, ordered for earliest first ST ----
            # wqk col chunks: 0=q01, 1=q23, 2=k01, 3=k23
            def dma_wqk(mf, k0=0, k1=KC):
                nc.sync.dma_start(
                    wqk_sb[:, k0:k1, mf * 128 : (mf + 1) * 128],
                    wqk_d[k0 * 128 : k1 * 128, mf * 128 : (mf + 1) * 128].rearrange(
                        "(kc p) c -> p kc c", p=128
                    ),
                )

            def dma_xT(nt, k0=0, k1=KC):
                nts = slice(nt * 512, (nt + 1) * 512)
                nc.sync.dma_start(
                    xT_sb[:, k0:k1, nts],
                    xT_d[k0 * 128 : k1 * 128, nts].rearrange(
                        "(kc p) c -> p kc c", p=128
                    ),
                )

            # first pieces sliced fine: the first qk matmuls (subtile deps)
            # start as soon as the first ~0.4MB lands
            dma_wqk(2, 0, 4)
            dma_xT(0, 0, 2)
            dma_xT(0, 2, 4)
            dma_wqk(2, 4, 8)
            dma_xT(0, 4, 6)
            dma_xT(0, 6, 8)
            dma_wqk(0)
            nc.sync.dma_start(
                wv_sb[:], wv_d.rearrange("(kc p) c -> p kc c", p=128)
            )
            dma_xT(1)
            dma_xT(2)
            dma_xT(3)
            dma_wqk(1)
            dma_wqk(3)
            for c2 in range(2):
                nc.sync.dma_start(wp_sb[:, c2, :], wp_d[c2 * 128 : (c2 + 1) * 128, :])

            # ---- emission helpers ----
            def qk_chunk(mf, nt, ps_pool=None):
                """(x @ Wqk)^T feat chunk mf, token chunk nt. 8 matmuls."""
                if ps_pool is None:
                    ps = fillps.tile([128, 512], f32, tag="fp", name="fp")
                else:
                    ps = ps_pool.tile([128, 512], f32, tag="st", name="st")
                for kc in range(KC):
                    nc.tensor.matmul(
                        ps,
                        wqk_sb[:, kc, mf * 128 : (mf + 1) * 128],
                        xT_sb[:, kc, nt * 512 : (nt + 1) * 512],
                        start=(kc == 0),
                        stop=(kc == KC - 1),
                    )
                nts = slice(nt * 512, (nt + 1) * 512)
                if mf < 2:
                    nc.vector.tensor_copy(qkT_sb[:, mf, nts], ps)
                else:
                    h0, h1 = 2 * (mf - 2), 2 * (mf - 2) + 1
                    nc.vector.tensor_copy(kTp_sb[0:64, h0, nts], ps[0:64, :])
                    nc.vector.tensor_copy(kTp_sb[64:128, h1, nts], ps[64:128, :])

            def v_chunk(t):
                """v = x @ Wv for token(=key) chunk t, all heads. 8 matmuls."""
                ps = fillps.tile([128, 512], f32, tag="fp", name="fp")[:, : HC * D]
                for kc in range(KC):
                    nc.tensor.matmul(
                        ps,
                        xT_sb[:, kc, t * 128 : (t + 1) * 128],
                        wv_sb[:, kc, :],
                        start=(kc == 0),
                        stop=(kc == KC - 1),
                    )
                nc.vector.tensor_copy(
                    v_sb[:, t, :, 0:D], ps.rearrange("p (h d) -> p h d", h=HC)
                )

            def proj_chunk(t, nf, ps_pool=None):
                """partial[t*128:(t+1)*128, nf*512:(nf+1)*512] = out @ Wp."""
                if ps_pool is None:
                    ps = fillps.tile([128, 512], f32, tag="fp", name="fp")
                else:
                    ps = ps_pool.tile([128, 512], f32, tag="pv", name="pv")
                for c2 in range(2):
                    nc.tensor.matmul(
                        ps,
                        outT_sb[:, c2, t * 128 : (t + 1) * 128],
                        wp_sb[:, c2, nf * 512 : (nf + 1) * 512],
                        start=(c2 == 0),
                        stop=(c2 == 1),
                    )
                sg = stage.tile([128, 512], f32, tag="sg", name="sg")
                nc.vector.tensor_copy(sg, ps)
                nc.sync.dma_start(
                    out_d[t * 128 : (t + 1) * 128, nf * 512 : (nf + 1) * 512], sg
                )

            # ---- static fill schedule (EDF with per-step budget) ----
            # unit: (cost_ns, release_step, deadline_step, emit_fn)
            # deadlines are STRICT: the unit's consumer is at step dl+1 (or
            # later), so placing at step <= dl can never put a fill after
            # the PE instruction that waits on it (emission-order deadlock).
            units = []
            # deadlines sit ~2 steps before the consuming ST so the DVE copy
            # into qkT/kTp has fully drained when the ST issues (dl at
            # consumer-1 left ~1us stalls entering each new block)
            qk_dl = {
                (2, 1): 1, (2, 2): 3, (2, 3): 5,    # k01 feeds ST(2i) in (0,0)
                (0, 1): 6, (0, 2): 21, (0, 3): 37,  # q01 feeds (0,rc) first ST
                (3, 0): 13, (3, 1): 15, (3, 2): 17, (3, 3): 19,  # k23
                (1, 0): 13, (1, 1): 29, (1, 2): 45, (1, 3): 53,  # q23
            }
            # stagger far-deadline releases so the EDF doesn't drain the fill
            # supply early and starve mid-kernel steps
            qk_rel = {
                (2, 1): 0, (2, 2): 0, (2, 3): 0,
                (0, 1): 0, (0, 2): 12, (0, 3): 33,
                (3, 0): 5, (3, 1): 5, (3, 2): 6, (3, 3): 6,
                (1, 0): 5, (1, 1): 20, (1, 2): 40, (1, 3): 49,
            }
            for (mf, nt), dl in qk_dl.items():
                units.append(
                    (1706, qk_rel[(mf, nt)], dl,
                     lambda mf=mf, nt=nt: qk_chunk(mf, nt))
                )
            for t in range(16):
                dl = min(t // 2, 6)  # PV(kc=t) at step t//2+1 (in-step at 7)
                units.append((853, 0, dl, lambda t=t: v_chunk(t)))
            # proj: releases staggered across the supply window; rc2 keeps 4
            # units reserved for the tail (rel 64) so the PE has warm work
            # during the last block's rescale chain
            for rc in range(4):
                for i, (t, nf) in enumerate(
                    (t, nf) for t in range(4 * rc, 4 * rc + 4) for nf in range(2)
                ):
                    rel, dl = {0: 25 + i, 1: 41 + i, 2: 57 + i, 3: 66}[rc], 99
                    if rc == 2 and i >= 4:
                        # 1 unit pinned to step 63 (cover the last exp wait),
                        # 3 go to the tail to overlap the final rescale chain
                        rel, dl = (63, 63) if i < 5 else (64, 99)
                    units.append(
                        (427, rel, dl,
                         lambda t=t, nf=nf, **kw: proj_chunk(t, nf, **kw))
                    )

            # attention PE cost per step (ns): first step of a block has no
            # PV (skew), last step has ST + 2 PVs
            def attn_cost(s):
                k = s % 8
                return 853 if k == 0 else (2559 if k == 7 else 1706)

            slots = defaultdict(list)  # step -> [unit...]
            pending = sorted(units, key=lambda u: (u[2], u[1]))
            placed_cost = defaultdict(int)
            unplaced = list(pending)
            for s in range(64):
                ready = [u for u in unplaced if u[1] <= s]
                ready.sort(key=lambda u: u[2])
                # forced: deadline at or before this step
                while ready and ready[0][2] <= s:
                    u = ready.pop(0)
                    slots[s].append(u)
                    placed_cost[s] += u[0]
                    unplaced.remove(u)
                while ready and placed_cost[s] + attn_cost(s) < TARGET_NS:
                    u = ready.pop(0)
                    slots[s].append(u)
                    placed_cost[s] += u[0]
                    unplaced.remove(u)
            tail_units = unplaced

            # ---- attention blocks ----
            pv_tiles = {}
            ex_pend = []  # (heads, kcs, pv, ex{h}) awaiting PV emission

            def emit_st_head(hp, rc, kc2, h, stp):
                """Both j-halves of one head's S^T step, then its exp."""
                for j in range(2):
                    kc = 2 * kc2 + j
                    nc.tensor.matmul(
                        stp[h][:, j * 512 : (j + 1) * 512],
                        kTp_sb[:, h, kc * 128 : (kc + 1) * 128],
                        qkT_sb[:, hp, rc * 512 : (rc + 1) * 512],
                        start=True,
                        stop=True,
                    )
                ex = expool.tile([128, 1024], in_dt, tag="ex", name="ex")
                nc.scalar.activation(
                    ex, stp[h], mybir.ActivationFunctionType.Exp, scale=SCALE
                )
                return ex

            def emit_pv_head(pv, h, kc2, ex):
                for j in range(2):
                    kc = 2 * kc2 + j
                    nc.tensor.matmul(
                        pv[h][: D + 1, :],
                        v_sb[:, kc, h, :],
                        ex[:, j * 512 : (j + 1) * 512],
                        start=(kc == 0),
                        stop=(kc == 15),
                    )

            def emit_rescale(hp, rc, heads, pv):
                recs, rbcs, dens = {}, {}, {}
                for h in heads:
                    if RECIP == "approx":
                        # recip_approx_fast breaks on partition-offset input;
                        # stage the denominator row at partition 0 first.
                        dens[h] = dpool.tile([1, 512], f32, tag="dcp", name="dcp")
                        nc.vector.tensor_copy(dens[h], pv[h][D : D + 1, :])
                for h in heads:
                    recs[h] = dpool.tile([1, 512], f32, tag="den", name="den")
                    if RECIP == "approx":
                        nc.vector.reciprocal_approx_fast(recs[h], dens[h])
                    else:
                        nc.vector.reciprocal(recs[h], pv[h][D : D + 1, :])
                for h in heads:
                    rbcs[h] = dpool.tile([64, 512], f32, tag="rbc", name="rbc")
                    nc.gpsimd.partition_broadcast(rbcs[h], recs[h])
                for h in heads:
                    hb = (h % 2) * 64
                    nc.vector.tensor_tensor(
                        out=outT_sb[hb : hb + 64, hp, rc * 512 : (rc + 1) * 512],
                        in0=pv[h][0:D, :],
                        in1=rbcs[h][:],
                        op=mybir.AluOpType.mult,
                    )

            # prefix: minimum to start attention (second chunk on the idle
            # stps pool so it doesn't serialize on fillps behind the first)
            qk_chunk(2, 0)               # k01 keys 0:512
            qk_chunk(0, 0, ps_pool=stps)  # q01 rows 0:512

            def pop_pv(pv):
                h, k2, ex = ex_pend.pop(0)
                emit_pv_head(pv, h, k2, ex)

            rescale_after = None  # (hp, rc, heads, pv) from previous block
            for s in range(64):
                bi, kc2 = s // 8, s % 8
                hp, rc = BLOCKS[bi]
                heads = (2 * hp, 2 * hp + 1)
                if kc2 == 0:
                    pv_tiles[(hp, rc)] = {
                        h: pvps.tile([128, 512], f32, tag="pv", name="pv")
                        for h in heads
                    }
                pv = pv_tiles[(hp, rc)]
                # round-robin fills into 4 gaps between ST/PV half-groups so
                # no two fills are adjacent (fillps bufs=1: back-to-back
                # fills stall on the PSUM->SBUF copy of the previous one)
                fp = [[], [], [], []]
                if kc2 == 0:
                    # block boundary: ST waits on the previous step's exp via
                    # stp recycle — put all fills before the STs
                    fills = slots.get(s, [])
                    fp[0], fp[1] = fills[0::2], fills[1::2]
                else:
                    for i, u in enumerate(slots.get(s, [])):
                        fp[i % 4].append(u)
                stp = {
                    h: stps.tile([128, 1024], f32, tag="st", name="st") for h in heads
                }
                for u in fp[0]:
                    u[3]()
                ex_pend.append(
                    (heads[0], kc2, emit_st_head(hp, rc, kc2, heads[0], stp))
                )
                for u in fp[1]:
                    u[3]()
                ex_pend.append(
                    (heads[1], kc2, emit_st_head(hp, rc, kc2, heads[1], stp))
                )
                for u in fp[2]:
                    u[3]()
                if kc2 == 0:
                    # no PVs this step (block boundary); finish prev block
                    if rescale_after is not None:
                        emit_rescale(*rescale_after)
                        rescale_after = None
                    for u in fp[3]:
                        u[3]()
                else:
                    pop_pv(pv)
                    for u in fp[3]:
                        u[3]()
                    pop_pv(pv)
                    if kc2 == 7:
                        pop_pv(pv)  # PV(kc2=7) in-step
                        pop_pv(pv)
                        rescale_after = (hp, rc, heads, pv)
            # tail: last block rescale, then remaining fills (proj rc=3).
            # The attention psum pools are free now — rotate proj psums
            # across pvps+fillps so the units pipeline instead of each
            # matmul waiting on the previous unit's PSUM->SBUF copy.
            # three reserved proj2 units first (independent of the last
            # rescale) keep the PE warm while the ~4us rescale chain runs;
            # the 3rd reuses fillps once the 1st unit's copy has drained
            for u in tail_units[:1]:
                u[3]()  # fillps
            for u in tail_units[1:2]:
                u[3](ps_pool=pvps)  # the one free pvps buffer
            for u in tail_units[2:3]:
                u[3]()  # fillps again
            emit_rescale(*rescale_after)
            for u in tail_units[3:]:
                u[3](ps_pool=pvps)
    nc.compile()
    return nc


def make_in_maps(x, w_qkv, w_proj):
    in_maps = []
    for core in range(NCORES):
        b, g = core // 4, core % 4
        qs = slice(g * 256, (g + 1) * 256)
        in_maps.append(
            {
                "xT": _prep(x[b].T),
                "wqk": _prep(
                    np.concatenate(
                        [w_qkv[:, qs], w_qkv[:, C + g * 256 : C + (g + 1) * 256]],
                        axis=1,
                    )
                ),
                "wv": _prep(w_qkv[:, 2 * C + g * 256 : 2 * C + (g + 1) * 256]),
                "wp": _prep(w_proj[qs, :]),
            }
        )
    return in_maps


def run_hw(x, w_qkv, w_proj, b_proj, trace=False):
    """Returns (full output [2, 2048, 1024] f32, exec_time_ns or None)."""
    in_maps = make_in_maps(x, w_qkv, w_proj)
    nc = build_nc()
    r = run_bass_kernel_spmd(nc, in_maps, core_ids=list(range(NCORES)), trace=trace)
    full = np.zeros((B, N, C), np.float32)
    for core in range(NCORES):
        full[core // 4] += r.results[core]["out"]
    full += np.asarray(b_proj, np.float32)[None, None, :]
    return full, r.exec_time_ns


def kernel(**inputs):
    x = np.asarray(inputs["x"], np.float32)
    w_qkv = np.asarray(inputs["w_qkv"], np.float32)
    w_proj = np.asarray(inputs["w_proj"], np.float32)
    b_proj = np.asarray(inputs["b_proj"], np.float32)
    out, _ = run_hw(x, w_qkv, w_proj, b_proj, trace=False)
    return out
